# revision 1
# baseline (speedup 1.0000x reference)
"""Criss-cross (CCNet) sparse attention kernel for Trainium2, 8-core data-parallel.

Problem (hardcoded): B=8, CQ=64, CV=512, H=W=128, fp32 I/O.
Per core: one image.  reference:
    energy_H[i,w,j] = sum_c q[c,i,w] k[c,j,w]   (diag i==j masked -inf)
    energy_W[i,w,j] = sum_c q[c,i,w] k[c,i,j]
    att = softmax(concat(energy_H, energy_W), axis=j)  (256-way per pixel)
    out[c,i,w] = sum_j v[c,j,w] att_H[i,w,j] + sum_j v[c,i,j] att_W[i,w,j]

Kernel strategy (everything c-major so row/col passes share one accumulator):
  - q/k cast to fp16 on load (halves SBUF + load bytes; energy error is
    negligible vs the bf16 attention quantization — verified vs reference)
  - E_W per row i:  psum[j,w] = k_i^T q_i (K=64); E_H per col w: psum[j,i]
  - att = exp(E - 40) -> bf16 stored [j, pix]; diagonal of att_H zeroed by a
    DVE multiply with a (1-I) mask before the denominator accumulation
  - denominators: basis-matmul psum accumulation (2 parallel accumulators
    per map) -> dn[i,w]; reciprocal; attention scaled in place by 1/dn via
    rank-1 broadcast matmuls (from a flattened [33, PIX] recip) + DVE mult
  - v cast fp32->bf16 into an i-major per-chunk DRAM scratch [ck, i, c, j]
    (512B source runs; nat re-load becomes fully contiguous)
  - per 128-channel chunk: nat_ch [i,(c,j)] single-descriptor-per-partition
    load (col-pass lhsT), vt_ch [j,(i,c)] xbar-transpose load (row-pass
    lhsT); PV matmuls produce [c_chunk, pix] psum tiles; row tiles
    ACT-copied, col tiles DVE-added (in i-halves so each out half DMAs
    while the other computes); output DMA lands directly in [c,i,w] fp32.
"""

import threading

import numpy as np

CQ, CV, H, W = 64, 512, 128, 128
PIX = H * W
B = 8
EXP_BIAS = -40.0
CHUNK = 128
N_CHUNKS = CV // CHUNK


def build_nc():
    import concourse.mybir as mybir
    import concourse.tile as tile
    from concourse import bacc
    from concourse.masks import make_identity

    f32 = mybir.dt.float32
    bf16 = mybir.dt.bfloat16
    fp16 = mybir.dt.float16
    Exp = mybir.ActivationFunctionType.Exp
    add = mybir.AluOpType.add
    mult = mybir.AluOpType.mult

    nc = bacc.Bacc(None, target_bir_lowering=False)

    with tile.TileContext(nc) as tc:
        with (
            tc.tile_pool(name="dram", bufs=1, space="DRAM") as dram,
            tc.tile_pool(name="attp", bufs=1) as attp,
        ):
            q_d = dram.tile((CQ, H, W), f32, kind="ExternalInput", name="q", uniquify=False)
            k_d = dram.tile((CQ, H, W), f32, kind="ExternalInput", name="k", uniquify=False)
            v_d = dram.tile((CV, H, W), f32, kind="ExternalInput", name="v", uniquify=False)
            o_d = dram.tile((CV, H, W), f32, kind="ExternalOutput", name="o", uniquify=False)
            # i-major per-chunk scratch: [chunk, i, c_in_chunk, j]
            vbf_d = dram.tile((N_CHUNKS, H, CHUNK, W), bf16, kind="Internal", name="vbf")

            # att_W[j, i*W + w] ; att_H[j, w*H + i]  (bf16, denominator-scaled)
            att_W = attp.tile([128, PIX], bf16)
            att_H = attp.tile([128, PIX], bf16)

            # v cast kicked off first (overlaps phase 1); per chunk+half to
            # stay under the SWDGE descriptor limit
            for ck in range(N_CHUNKS):
                for ih in range(2):
                    nc.gpsimd.dma_start(
                        out=vbf_d[ck, ih * 64:(ih + 1) * 64],
                        in_=v_d[ck * CHUNK:(ck + 1) * CHUNK,
                                ih * 64:(ih + 1) * 64, :].rearrange("c i j -> i c j"),
                    )

            # ---- phase 1: energies, exp, denominators, att scaling
            with (
                tc.tile_pool(name="const", bufs=1) as constp,
                tc.tile_pool(name="dnp", bufs=1) as dnp,
                tc.tile_pool(name="rflat", bufs=1) as rflatp,
            ):
                ident = constp.tile([128, 128], f32)
                make_identity(nc, ident[:])
                # maskM4[j, (d, i)] = 0 on j==i diagonal else 1 (4 copies)
                ident_bf = constp.tile([128, 128], bf16)
                nc.vector.tensor_copy(ident_bf[:], ident[:])
                maskM4 = constp.tile([128, 512], bf16)
                for d in range(4):
                    nc.vector.tensor_scalar(
                        maskM4[:, d * 128:(d + 1) * 128], ident_bf[:],
                        -1.0, 1.0, op0=mult, op1=add,
                    )
                # Z[j, x] = 1.0 iff x == 128 (basis lhsT: Z[:,128-i:256-i])
                zb = constp.tile([128, 256], bf16)
                nc.vector.memset(zb[:], 0.0)
                nc.vector.memset(zb[:, 128:129], 1.0)
                ones1 = constp.tile([33, 128], f32)
                nc.vector.memset(ones1[:], 1.0)
                bias_t = constp.tile([128, 1], f32)
                nc.vector.memset(bias_t[:], EXP_BIAS)
                # recip maps flattened to partitions 0 (i,w-order) and 32
                # (w,i-order) so rank-1 rhs reads have legal base partitions;
                # one tile so only 64KB/partition is reserved
                r_fl = rflatp.tile([33, PIX], f32, name="r_fl")

                with (
                    tc.tile_pool(name="qk", bufs=1) as qkp,
                    tc.tile_pool(name="pse", bufs=4, space="PSUM") as pse,
                    tc.tile_pool(name="psdn", bufs=1, space="PSUM") as psdn,
                ):
                    q_sb = qkp.tile([CQ, H, W], fp16)
                    k_sb = qkp.tile([CQ, H, W], fp16)
                    for r0 in range(0, H, 32):
                        nc.gpsimd.dma_start(q_sb[:, r0:r0 + 32, :], q_d[:, r0:r0 + 32, :])
                        nc.gpsimd.dma_start(k_sb[:, r0:r0 + 32, :], k_d[:, r0:r0 + 32, :])

                    # two parallel accumulators per map halve the serial
                    # psum-accumulation chains; combined after the loops
                    dnW_ps = [psdn.tile([128, 128], f32, name=f"dnW_ps{a}") for a in range(2)]
                    dnH_ps = [psdn.tile([128, 128], f32, name=f"dnH_ps{a}") for a in range(2)]

                    for i0 in range(0, H, 4):
                        pe = pse.tile([128, 512], f32, name="pe_row", tag="pe")
                        for d in range(4):
                            i = i0 + d
                            nc.tensor.matmul(
                                pe[:, d * 128:(d + 1) * 128],
                                lhsT=k_sb[:, i, :], rhs=q_sb[:, i, :],
                                start=True, stop=True,
                            )
                        nc.scalar.activation(
                            att_W[:, i0 * W:(i0 + 4) * W], pe[:], Exp, bias=bias_t[:]
                        )
                        for d in range(4):
                            i = i0 + d
                            nc.tensor.matmul(
                                dnW_ps[i % 2][:], lhsT=zb[:, 128 - i:256 - i],
                                rhs=att_W[:, i * W:(i + 1) * W],
                                start=(i < 2), stop=(i >= H - 2),
                            )
                    for w0 in range(0, W, 4):
                        pe = pse.tile([128, 512], f32, name="pe_col", tag="pe")
                        for d in range(4):
                            w = w0 + d
                            nc.tensor.matmul(
                                pe[:, d * 128:(d + 1) * 128],
                                lhsT=k_sb[:, :, w], rhs=q_sb[:, :, w],
                                start=True, stop=True,
                            )
                        nc.scalar.activation(
                            att_H[:, w0 * H:(w0 + 4) * H], pe[:], Exp, bias=bias_t[:]
                        )
                        sl = att_H[:, w0 * H:(w0 + 4) * H]
                        nc.vector.tensor_tensor(sl, sl, maskM4[:], op=mult)
                        for d in range(4):
                            w = w0 + d
                            nc.tensor.matmul(
                                dnH_ps[w % 2][:], lhsT=zb[:, 128 - w:256 - w],
                                rhs=att_H[:, w * H:(w + 1) * H],
                                start=(w < 2), stop=(w >= W - 2),
                            )

                    # dn_iw = dnW + dnH^T ; dn_wi = dnH + dnW^T ; reciprocals
                    dnW_sb = dnp.tile([128, 128], f32)
                    nc.vector.tensor_copy(dnW_sb[:], dnW_ps[0][:])
                    nc.vector.tensor_tensor(dnW_sb[:], dnW_sb[:], dnW_ps[1][:], op=add)
                    dnH_sb = dnp.tile([128, 128], f32)
                    nc.vector.tensor_copy(dnH_sb[:], dnH_ps[0][:])
                    nc.vector.tensor_tensor(dnH_sb[:], dnH_sb[:], dnH_ps[1][:], op=add)
                    t1 = pse.tile([128, 128], f32, name="t1", tag="pe")
                    nc.tensor.transpose(t1[:], dnW_sb[:], ident[:])  # [w, i]
                    t2 = pse.tile([128, 128], f32, name="t2", tag="pe")
                    nc.tensor.transpose(t2[:], dnH_sb[:], ident[:])  # [i, w]
                    r_iw = dnp.tile([128, 128], f32)
                    nc.vector.tensor_tensor(r_iw[:], t2[:], dnW_sb[:], op=add)
                    nc.vector.reciprocal(r_iw[:], r_iw[:])
                    r_wi = dnp.tile([128, 128], f32)
                    nc.vector.tensor_tensor(r_wi[:], t1[:], dnH_sb[:], op=add)
                    nc.vector.reciprocal(r_wi[:], r_wi[:])
                    nc.sync.dma_start(r_fl[0:1, :], r_iw[:])
                    nc.sync.dma_start(r_fl[32:33, :], r_wi[:])

                # ---- att scaling: att *= 1/dn (pixel-wise, bcast over j)
                with tc.tile_pool(name="psr", bufs=2, space="PSUM") as psr:
                    for i0 in range(0, H, 4):
                        pr = psr.tile([128, 512], f32, name="pr_w")
                        for d in range(4):
                            i = i0 + d
                            nc.tensor.matmul(
                                pr[:, d * 128:(d + 1) * 128],
                                lhsT=ones1[0:1, :], rhs=r_fl[0:1, i * W:(i + 1) * W],
                                start=True, stop=True,
                            )
                        sl = att_W[:, i0 * W:(i0 + 4) * W]
                        nc.vector.tensor_tensor(sl, sl, pr[:], op=mult)
                    for w0 in range(0, W, 4):
                        pr = psr.tile([128, 512], f32, name="pr_h")
                        for d in range(4):
                            w = w0 + d
                            nc.tensor.matmul(
                                pr[:, d * 128:(d + 1) * 128],
                                lhsT=ones1[32:33, :], rhs=r_fl[32:33, w * H:(w + 1) * H],
                                start=True, stop=True,
                            )
                        sl = att_H[:, w0 * H:(w0 + 4) * H]
                        nc.vector.tensor_tensor(sl, sl, pr[:], op=mult)

            # ---- phase 2: PV, merge, output
            with (
                tc.tile_pool(name="natp", bufs=1) as natp,
                tc.tile_pool(name="vtp", bufs=1) as vtp,
                tc.tile_pool(name="outp", bufs=1) as outp,
                tc.tile_pool(name="psrow", bufs=4, space="PSUM") as psrow,
                tc.tile_pool(name="pscol", bufs=4, space="PSUM") as pscol,
            ):
                for ck in range(N_CHUNKS):
                    c0 = ck * CHUNK
                    nat_ch = natp.tile([128, CHUNK, 128], bf16, name="nat_ch")  # [i, c, j]
                    nc.sync.dma_start(
                        nat_ch[:].rearrange("i c j -> i (c j)"),
                        vbf_d[ck].rearrange("i c j -> i (c j)"),
                    )
                    vt_ch = vtp.tile([128, 128, CHUNK], bf16, name="vt_ch")  # [j, i, c]
                    nc.sync.dma_start(
                        vt_ch[:].rearrange("j i c -> j (i c)"),
                        vbf_d[ck].rearrange("i c j -> (i c) j"),
                        transpose=True,
                    )
                    out_ch = outp.tile([128, H, W], f32, name="out_ch")  # [c, i, w]

                    for i0 in range(0, H, 4):
                        pb = psrow.tile([128, 512], f32, name="pb_row")
                        for d in range(4):
                            i = i0 + d
                            nc.tensor.matmul(
                                pb[:, d * 128:(d + 1) * 128],
                                lhsT=vt_ch[:, i, :],
                                rhs=att_W[:, i * W:(i + 1) * W],
                                start=True, stop=True,
                            )
                        nc.scalar.copy(
                            out_ch[:, i0:i0 + 4, :].rearrange("c a b -> c (a b)"),
                            pb[:],
                        )
                    # col pass in i-halves: each out half DMAs while the other
                    # half's adds still run
                    for ih in range(2):
                        iofs = ih * 64
                        for w0 in range(0, W, 8):
                            pb = pscol.tile([128, 512], f32, name="pb_col")
                            for d in range(8):
                                w = w0 + d
                                nc.tensor.matmul(
                                    pb[:, d * 64:(d + 1) * 64],
                                    lhsT=nat_ch[:, :, w],
                                    rhs=att_H[:, w * H + iofs:w * H + iofs + 64],
                                    start=True, stop=True,
                                )
                            # out_ch[c, iofs+i, w0+d] += pb[c, d, i]
                            ov = out_ch[:, iofs:iofs + 64, w0:w0 + 8].transpose([0, 2, 1])
                            pb3 = pb[:].rearrange("c (d i) -> c d i", d=8)
                            nc.vector.tensor_tensor(ov, ov, pb3, op=add)
                        nc.scalar.dma_start(
                            o_d[c0:c0 + CHUNK, iofs:iofs + 64, :],
                            out_ch[:, iofs:iofs + 64, :],
                        )

    nc.compile()
    return nc


_CACHE = {}
_LOCK = threading.Lock()


def _get_nc():
    with _LOCK:
        if "nc" not in _CACHE:
            _CACHE["nc"] = build_nc()
        return _CACHE["nc"]


def kernel(proj_query: np.ndarray, proj_key: np.ndarray, proj_value: np.ndarray,
           trace: bool = False):
    from concourse.bass_utils import run_bass_kernel_spmd

    q = np.ascontiguousarray(np.asarray(proj_query, dtype=np.float32))
    k = np.ascontiguousarray(np.asarray(proj_key, dtype=np.float32))
    v = np.ascontiguousarray(np.asarray(proj_value, dtype=np.float32))
    assert q.shape == (B, CQ, H, W) and v.shape == (B, CV, H, W)

    nc = _get_nc()
    in_maps = [{"q": q[b], "k": k[b], "v": v[b]} for b in range(B)]
    res = run_bass_kernel_spmd(nc, in_maps, core_ids=list(range(B)), trace=trace)
    out = np.stack([res.results[b]["o"] for b in range(B)], axis=0)
    if trace:
        kernel.last_exec_time_ns = res.exec_time_ns
        kernel.last_results = res
    return out


if __name__ == "__main__":
    nc = build_nc()
    print("build ok:", nc)

